# revision 1
# baseline (speedup 1.0000x reference)
"""Trainium2 Bass kernel for nn_PointerAttention (head-mean pointer logits).

Reference computation (B=4, T=2048, S=4096, D=512, H=8, HD=64):
    q = query @ q_w.T + q_b
    k = keys  @ k_w.T + k_b
    logits[b,t,s] = sum_d q[b,t,d] * k[b,s,d] / (H * sqrt(HD))   # = /64
    logits = where(mask[b,s], -inf, logits)

Algebraic refactor (all folding done on host in float64):
    Q = X Wq^T + 1 bq^T ;  K = Y Wk^T + 1 bk^T
    Q K^T = X (Wq^T Wk) Y^T + 1 (Y Wk^T bq)^T + (X Wq^T bk + bq.bk) 1^T
    Let  M = Wq^T Wk / 64          [D, D]
         v = Wk^T bq / 64          [D]     (per-partition bias of stage 1)
         w = (X (Wq^T bk) + bq.bk)/64  [T] per batch (per-partition bias, stage 2)
    Then out = (X M + 1 v^T) Y^T + w 1^T
       stage 1 (device): Q2T[e,t] = sum_c M[c,e] xT[c,t] + v[e]
       stage 2 (device): out[t,s] = sum_e Q2T[e,t] yT[e,s] + w[t]
    where xT = query[b].T and yT = keys[b].T are RAW inputs — only one
    projection-sized matmul remains and the K-side projection disappears.

Sharding: 8 cores = 4 batches x 2 key-column halves. Each core computes
out[b, :, half] = [2048, 2048] (16 MiB). No collectives.

Matmuls run in float32r (1 cycle/row on TRN2 PE vs 4 for float32).
"""

import os
from contextlib import ExitStack

import numpy as np

import concourse.bass as bass  # noqa: F401  (bass types used via tile/bacc)
import concourse.tile as tile
from concourse import bacc, mybir
from concourse.bass_utils import run_bass_kernel_spmd

# Problem dims (hardcoded; harness contract)
B, T, S, D = 4, 2048, 4096, 512
SCALE = 64.0  # N_HEADS * sqrt(HEAD_DIM) = 8 * 8
N_CORES = 8
SHALF = S // 2  # keys columns per core
P = 128  # SBUF partitions
FD = 512  # matmul moving free dim == one fp32 PSUM bank
KC = D // P  # contraction chunks (4)
NT_TILES = T // P  # output row tiles per core (16)
NS_CHUNKS = SHALF // FD  # output col chunks per core (4)
NT_CHUNKS = T // FD  # stage-1 moving chunks (4)

_NC_CACHE: dict = {}

# experiment toggles (timing A/B only; defaults are the shipped config)
K_WARMUP = os.environ.get("K_WARMUP", "0") == "1"
K_OUT_RING = os.environ.get("K_OUT_RING", "act")
K_NO_OUT = os.environ.get("K_NO_OUT", "0") == "1"   # timing ablation only
K_EVICT = os.environ.get("K_EVICT", "split")        # split | act | dve
K_SBLK = int(os.environ.get("K_SBLK", "2"))  # s-chunks per stage-2 pass
K_INCH = int(os.environ.get("K_INCH", "512"))  # input DMA chunk columns
K_MMORD = os.environ.get("K_MMORD", "es")  # ei: e-inner | es: e-outer/s-inner


def _alloc(ctx: ExitStack, tc):
    f32 = mybir.dt.float32
    f32r = mybir.dt.float32r
    persist = ctx.enter_context(tc.tile_pool(name="persist", bufs=1))
    psum = ctx.enter_context(tc.tile_pool(name="psum", bufs=8, space="PSUM"))
    ostage = ctx.enter_context(tc.tile_pool(name="ostage", bufs=6))
    tiles = {
        "psum": psum,
        "ostage": ostage,
        "m": [persist.tile([P, D], f32r, tag=f"m{c}", name=f"m{c}") for c in range(KC)],
        "x": [persist.tile([P, T], f32r, tag=f"x{c}", name=f"x{c}") for c in range(KC)],
        "y": [
            persist.tile([P, SHALF], f32r, tag=f"y{e}", name=f"y{e}")
            for e in range(KC)
        ],
        "q2": [
            persist.tile([P, T], f32r, tag=f"q2{e}", name=f"q2{e}") for e in range(KC)
        ],
        "v": persist.tile([P, KC], f32, tag="v", name="vt"),
        "w": persist.tile([P, NT_TILES], f32, tag="w", name="wt"),
        "warm": persist.tile([P, 256], f32r, tag="warm", name="warm"),
        "warm_f32": persist.tile([P, 256], f32, tag="warm_f32", name="warm_f32"),
    }
    return tiles


def _emit_body(tiles, tc, xT, yT, m, v, w, out):
    nc = tc.nc
    ident = mybir.ActivationFunctionType.Identity
    psum, ostage = tiles["psum"], tiles["ostage"]
    m_t, x_t, y_t, q2_t = tiles["m"], tiles["x"], tiles["y"], tiles["q2"]
    v_t, w_t = tiles["v"], tiles["w"]

    nc.sync.dma_start(v_t[:], v[:])
    nc.sync.dma_start(w_t[:], w[:])

    # PE warmup: ~16 junk matmuls during the initial DMA wait so the HAM
    # clock-gate reaches 8/8 before the first real matmul.
    if K_WARMUP:
        warm = tiles["warm"]
        warm_f32 = tiles["warm_f32"]
        wps = tiles["psum"].tile(
            [P, 256], mybir.dt.float32, tag="wps", name="wps", bufs=1
        )
        nc.vector.memset(warm_f32[:], 0.0)
        nc.vector.tensor_copy(warm[:], warm_f32[:])
        for i in range(16):
            nc.tensor.matmul(
                wps[:], warm[:, 0:P], warm[:], start=(i == 0), stop=(i == 15)
            )

    # Input loads straight into float32r tiles (DRAM tensors are declared
    # f32r, so the DMACopy producer satisfies walrus' fp32r check).
    # Order = consumption order: M in stage-1 e-order, x in stage-1 n-order
    # (keeps stage 1 fed), then y in stage-2 s-order (s=0 cols of every
    # e-tile arrive first). Inputs ride the SP HWDGE ring; outputs ride the
    # ACT ring (separate FIFO, no head-of-line blocking between the two).
    for c in range(KC):
        nc.sync.dma_start(m_t[c][:], m[c * P:(c + 1) * P, :])
    for n in range(T // K_INCH):
        for c in range(KC):
            nc.sync.dma_start(
                x_t[c][:, n * K_INCH:(n + 1) * K_INCH],
                xT[c * P:(c + 1) * P, n * K_INCH:(n + 1) * K_INCH],
            )
    for s in range(SHALF // K_INCH):
        for e in range(KC):
            nc.sync.dma_start(
                y_t[e][:, s * K_INCH:(s + 1) * K_INCH],
                yT[e * P:(e + 1) * P, s * K_INCH:(s + 1) * K_INCH],
            )

    # Stage 1 chunk emitter: Q2T[e, t] = sum_c M[c,e] xT[c,t] + v[e]
    def stage1(n):
        if K_MMORD == "es":
            for e in range(KC):
                ps = psum.tile(
                    [P, FD], mybir.dt.float32, tag=f"ps{e}", name="ps", bufs=2
                )
                for c in range(KC):
                    nc.tensor.matmul(
                        ps[:],
                        m_t[c][:, e * P:(e + 1) * P],
                        x_t[c][:, n * FD:(n + 1) * FD],
                        start=(c == 0),
                        stop=(c == KC - 1),
                    )
                if K_EVICT == "act" or (K_EVICT == "split" and e % 2 == 0):
                    nc.scalar.activation(
                        q2_t[e][:, n * FD:(n + 1) * FD], ps[:], ident,
                        bias=v_t[:, e:e + 1],
                    )
                else:
                    nc.vector.tensor_scalar_add(
                        q2_t[e][:, n * FD:(n + 1) * FD], ps[:], v_t[:, e:e + 1]
                    )
            return
        for e in range(KC):
            ps = psum.tile([P, FD], mybir.dt.float32, tag="ps", name="ps", bufs=7)
            for c in range(KC):
                nc.tensor.matmul(
                    ps[:],
                    m_t[c][:, e * P:(e + 1) * P],
                    x_t[c][:, n * FD:(n + 1) * FD],
                    start=(c == 0),
                    stop=(c == KC - 1),
                )
            # eviction rounds to f32r for the stage-2 matmul; alternate engines
            if K_EVICT == "act" or (K_EVICT == "split" and e % 2 == 0):
                nc.scalar.activation(
                    q2_t[e][:, n * FD:(n + 1) * FD], ps[:], ident, bias=v_t[:, e:e + 1]
                )
            else:
                nc.vector.tensor_scalar_add(
                    q2_t[e][:, n * FD:(n + 1) * FD], ps[:], v_t[:, e:e + 1]
                )

    out_eng = nc.scalar if K_OUT_RING == "act" else nc.sync

    # Stage 2: out[t, s] = sum_e Q2T[e,t] yT[e,s] + w[t].
    # Two PE orders:
    #  ei: per (tt, s) accumulate over e (both matmul operands advance each
    #      MM), processed in s-blocks of K_SBLK so the first pass needs only
    #      part of y.
    #  es: per tt, e-outer / s-inner across 4 parallel PSUM banks — the
    #      stationary operand q2[e][:,tt] is reused by 4 consecutive MMs,
    #      which measures ~25% faster per MM on HW.
    def stage2_tile(sb, tt):
        ot = ostage.tile([P, K_SBLK * FD], mybir.dt.float32, tag="ot", name="ot")
        for j in range(K_SBLK):
            s = sb * K_SBLK + j
            ps = psum.tile([P, FD], mybir.dt.float32, tag="ps", name="ps", bufs=7)
            for e in range(KC):
                nc.tensor.matmul(
                    ps[:],
                    q2_t[e][:, tt * P:(tt + 1) * P],
                    y_t[e][:, s * FD:(s + 1) * FD],
                    start=(e == 0),
                    stop=(e == KC - 1),
                )
            if K_EVICT == "act" or (K_EVICT == "split" and (tt + s) % 2 == 0):
                nc.scalar.activation(
                    ot[:, j * FD:(j + 1) * FD], ps[:], ident, bias=w_t[:, tt:tt + 1]
                )
            else:
                nc.vector.tensor_scalar_add(
                    ot[:, j * FD:(j + 1) * FD], ps[:], w_t[:, tt:tt + 1]
                )
        last = sb == NS_CHUNKS // K_SBLK - 1 and tt == NT_TILES - 1
        if not K_NO_OUT or last:
            out_eng.dma_start(
                out[tt * P:(tt + 1) * P, sb * K_SBLK * FD:(sb + 1) * K_SBLK * FD],
                ot[:],
            )

    def stage2_tile_es(tt):
        ot = ostage.tile([P, SHALF], mybir.dt.float32, tag="ot", name="ot")
        pss = [
            psum.tile([P, FD], mybir.dt.float32, tag=f"ps{s}", name=f"ps{s}", bufs=2)
            for s in range(NS_CHUNKS)
        ]
        for e in range(KC):
            for s in range(NS_CHUNKS):
                nc.tensor.matmul(
                    pss[s][:],
                    q2_t[e][:, tt * P:(tt + 1) * P],
                    y_t[e][:, s * FD:(s + 1) * FD],
                    start=(e == 0),
                    stop=(e == KC - 1),
                )
        for s in range(NS_CHUNKS):
            if K_EVICT == "act" or (K_EVICT == "split" and (tt + s) % 2 == 0):
                nc.scalar.activation(
                    ot[:, s * FD:(s + 1) * FD], pss[s][:], ident,
                    bias=w_t[:, tt:tt + 1],
                )
            else:
                nc.vector.tensor_scalar_add(
                    ot[:, s * FD:(s + 1) * FD], pss[s][:], w_t[:, tt:tt + 1]
                )
        if not K_NO_OUT or tt == NT_TILES - 1:
            out_eng.dma_start(out[tt * P:(tt + 1) * P, :], ot[:])

    # PE program order: all of stage 1 (its span covers the y DMA window),
    # then stage 2.
    for n in range(NT_CHUNKS):
        stage1(n)
    if K_MMORD == "es":
        for tt in range(NT_TILES):
            stage2_tile_es(tt)
    else:
        for sb in range(NS_CHUNKS // K_SBLK):
            for tt in range(NT_TILES):
                stage2_tile(sb, tt)


def _build(reps: int = 1, loop_reps: int = 1):
    """Build + compile the per-core Bass program. reps>1 statically unrolls
    the whole body; loop_reps>1 wraps it in a runtime For_i loop (both are
    used only for timing measurements)."""
    key = (reps, loop_reps)
    if key in _NC_CACHE:
        return _NC_CACHE[key]
    nc = bacc.Bacc(trn_type="TRN2", target_bir_lowering=False, debug=False)
    f32 = mybir.dt.float32
    f32r = mybir.dt.float32r
    xT = nc.dram_tensor("xT", [D, T], f32r, kind="ExternalInput").ap()
    yT = nc.dram_tensor("yT", [D, SHALF], f32r, kind="ExternalInput").ap()
    m = nc.dram_tensor("m", [D, D], f32r, kind="ExternalInput").ap()
    v = nc.dram_tensor("v", [P, KC], f32, kind="ExternalInput").ap()
    w = nc.dram_tensor("w", [P, NT_TILES], f32, kind="ExternalInput").ap()
    out = nc.dram_tensor("out", [T, SHALF], f32, kind="ExternalOutput").ap()
    with tile.TileContext(nc) as tc:
        with ExitStack() as ctx:
            tiles = _alloc(ctx, tc)
            if loop_reps > 1:
                hint = (
                    mybir.EngineType.PE,
                    mybir.EngineType.Activation,
                    mybir.EngineType.DVE,
                    mybir.EngineType.SP,
                )
                with tc.For_i(0, loop_reps, 1, hint_engines=hint):
                    for _ in range(reps):
                        _emit_body(tiles, tc, xT, yT, m, v, w, out)
            else:
                for _ in range(reps):
                    _emit_body(tiles, tc, xT, yT, m, v, w, out)
    nc.compile()
    _NC_CACHE[key] = nc
    return nc


def _host_prep(query, keys, q_w, q_b, k_w, k_b):
    """Fold weights/biases on host (float64), build per-core input maps."""
    q_w64 = np.asarray(q_w, np.float64)
    k_w64 = np.asarray(k_w, np.float64)
    q_b64 = np.asarray(q_b, np.float64)
    k_b64 = np.asarray(k_b, np.float64)

    m_in = np.ascontiguousarray(((q_w64.T @ k_w64) / SCALE).astype(np.float32))
    v64 = (k_w64.T @ q_b64) / SCALE  # [D]
    v_in = np.ascontiguousarray(v64.astype(np.float32).reshape(KC, P).T)
    g = q_w64.T @ k_b64  # [D]
    cc = float(q_b64 @ k_b64)
    # w[b, t] = (query[b] @ g + bq.bk) / 64
    w_all = ((np.asarray(query, np.float64) @ g + cc) / SCALE).astype(np.float32)

    in_maps = []
    for i in range(N_CORES):
        b, h = divmod(i, N_CORES // B)
        in_maps.append(
            {
                "xT": np.ascontiguousarray(query[b].T),
                "yT": np.ascontiguousarray(keys[b, h * SHALF:(h + 1) * SHALF, :].T),
                "m": m_in,
                "v": v_in,
                "w": np.ascontiguousarray(w_all[b].reshape(NT_TILES, P).T),
            }
        )
    return in_maps


def _gather(results, mask):
    out = np.empty((B, T, S), np.float32)
    for i in range(N_CORES):
        b, h = divmod(i, N_CORES // B)
        out[b, :, h * SHALF:(h + 1) * SHALF] = results[i]["out"]
    if mask is not None and mask.any():
        out = np.where(mask[:, None, :], np.float32(-np.inf), out)
    return out


def kernel(query, keys, key_padding_mask, q_w, q_b, k_w, k_b):
    query = np.asarray(query, np.float32)
    keys = np.asarray(keys, np.float32)
    mask = np.asarray(key_padding_mask, bool)
    assert query.shape == (B, T, D) and keys.shape == (B, S, D)

    in_maps = _host_prep(query, keys, q_w, q_b, k_w, k_b)
    nc = _build(reps=1)
    res = run_bass_kernel_spmd(nc, in_maps, core_ids=list(range(N_CORES)))
    return _gather(res.results, mask)



# revision 8
# speedup vs baseline: 1.1216x; 1.1216x over previous
"""Trainium2 Bass kernel for nn_PointerAttention (head-mean pointer logits).

Reference computation (B=4, T=2048, S=4096, D=512, H=8, HD=64):
    q = query @ q_w.T + q_b
    k = keys  @ k_w.T + k_b
    logits[b,t,s] = sum_d q[b,t,d] * k[b,s,d] / (H * sqrt(HD))   # = /64
    logits = where(mask[b,s], -inf, logits)

Algebraic refactor (all folding done on host in float64):
    Q = X Wq^T + 1 bq^T ;  K = Y Wk^T + 1 bk^T
    Q K^T = X (Wq^T Wk) Y^T + 1 (Y Wk^T bq)^T + (X Wq^T bk + bq.bk) 1^T
    Let  M = Wq^T Wk / 64          [D, D]
         v = Wk^T bq / 64          [D]     (per-partition bias of stage 1)
         w = (X (Wq^T bk) + bq.bk)/64  [T] per batch (per-partition bias, stage 2)
    Then out = (X M + 1 v^T) Y^T + w 1^T
       stage 1 (device): Q2T[e,t] = sum_c M[c,e] xT[c,t] + v[e]
       stage 2 (device): out[t,s] = sum_e Q2T[e,t] yT[e,s] + w[t]
    where xT = query[b].T and yT = keys[b].T are RAW inputs — only one
    projection-sized matmul remains and the K-side projection disappears.

Sharding: 8 cores = 4 batches x 2 key-column halves. Each core computes
out[b, :, half] = [2048, 2048]. No collectives.

The whole device pipeline runs in float16 (inputs, Q2 intermediate, and
output; PSUM accumulation stays f32): fp16 matmul is 1 cycle/row on the
TRN2 PE (same as f32r) and halves DMA bytes — 13.1 MB/core vs 26.2 MB —
which is what matters in this memory-bound regime. fp16's 10-bit
mantissa keeps rel-to-scale error ~5e-4 (measured vs f64), far inside
the 2e-2 gate. The host upcasts the fp16 output to f32 after gather.
"""

import os
from contextlib import ExitStack

import numpy as np

import concourse.bass as bass  # noqa: F401  (bass types used via tile/bacc)
import concourse.tile as tile
from concourse import bacc, mybir
from concourse.bass_utils import run_bass_kernel_spmd

# Problem dims (hardcoded; harness contract)
B, T, S, D = 4, 2048, 4096, 512
SCALE = 64.0  # N_HEADS * sqrt(HEAD_DIM) = 8 * 8
N_CORES = 8
SHALF = S // 2  # keys columns per core
P = 128  # SBUF partitions
FD = 512  # matmul moving free dim == one fp32 PSUM bank
KC = D // P  # contraction chunks (4)
NT_TILES = T // P  # output row tiles per core (16)
NS_CHUNKS = SHALF // FD  # output col chunks per core (4)
NT_CHUNKS = T // FD  # stage-1 moving chunks (4)

_NC_CACHE: dict = {}

# experiment toggles (timing A/B only; defaults are the shipped config)
K_WARMUP = os.environ.get("K_WARMUP", "0") == "1"
K_OUT_RING = os.environ.get("K_OUT_RING", "act")
K_NO_OUT = os.environ.get("K_NO_OUT", "0") == "1"   # timing ablation only
K_EVICT = os.environ.get("K_EVICT", "split")        # split | act | dve
K_SBLK = int(os.environ.get("K_SBLK", "2"))  # s-chunks per stage-2 pass
K_INCH = int(os.environ.get("K_INCH", "512"))  # input DMA chunk columns
K_MMORD = os.environ.get("K_MMORD", "es")  # ei: e-inner | es: e-outer/s-inner


def _alloc(ctx: ExitStack, tc):
    f32 = mybir.dt.float32
    f16 = mybir.dt.float16
    persist = ctx.enter_context(tc.tile_pool(name="persist", bufs=1))
    psum = ctx.enter_context(tc.tile_pool(name="psum", bufs=8, space="PSUM"))
    ostage = ctx.enter_context(tc.tile_pool(name="ostage", bufs=6))
    tiles = {
        "psum": psum,
        "ostage": ostage,
        "m": [persist.tile([P, D], f16, tag=f"m{c}", name=f"m{c}") for c in range(KC)],
        "x": [persist.tile([P, T], f16, tag=f"x{c}", name=f"x{c}") for c in range(KC)],
        "y": [
            persist.tile([P, SHALF], f16, tag=f"y{e}", name=f"y{e}")
            for e in range(KC)
        ],
        "q2": [
            persist.tile([P, T], f16, tag=f"q2{e}", name=f"q2{e}") for e in range(KC)
        ],
        "v": persist.tile([P, KC], f32, tag="v", name="vt"),
        "w": persist.tile([P, NT_TILES], f32, tag="w", name="wt"),
        "warm": persist.tile([P, 256], f16, tag="warm", name="warm"),
        "warm_f32": persist.tile([P, 256], f32, tag="warm_f32", name="warm_f32"),
    }
    return tiles


def _emit_body(tiles, tc, xT, yT, m, v, w, out):
    nc = tc.nc
    ident = mybir.ActivationFunctionType.Identity
    psum, ostage = tiles["psum"], tiles["ostage"]
    m_t, x_t, y_t, q2_t = tiles["m"], tiles["x"], tiles["y"], tiles["q2"]
    v_t, w_t = tiles["v"], tiles["w"]

    nc.sync.dma_start(v_t[:], v[:])
    nc.sync.dma_start(w_t[:], w[:])

    # PE warmup: ~16 junk matmuls during the initial DMA wait so the HAM
    # clock-gate reaches 8/8 before the first real matmul.
    if K_WARMUP:
        warm = tiles["warm"]
        warm_f32 = tiles["warm_f32"]
        wps = tiles["psum"].tile(
            [P, 256], mybir.dt.float32, tag="wps", name="wps", bufs=1
        )
        nc.vector.memset(warm_f32[:], 0.0)
        nc.vector.tensor_copy(warm[:], warm_f32[:])
        for i in range(16):
            nc.tensor.matmul(
                wps[:], warm[:, 0:P], warm[:], start=(i == 0), stop=(i == 15)
            )

    # Input loads straight into float32r tiles (DRAM tensors are declared
    # f32r, so the DMACopy producer satisfies walrus' fp32r check).
    # Order = consumption order: M in stage-1 e-order, x in stage-1 n-order
    # (keeps stage 1 fed), then y in stage-2 s-order (s=0 cols of every
    # e-tile arrive first). Inputs ride the SP HWDGE ring; outputs ride the
    # ACT ring (separate FIFO, no head-of-line blocking between the two).
    for c in range(KC):
        nc.sync.dma_start(m_t[c][:], m[c * P:(c + 1) * P, :])
    for n in range(T // K_INCH):
        for c in range(KC):
            nc.sync.dma_start(
                x_t[c][:, n * K_INCH:(n + 1) * K_INCH],
                xT[c * P:(c + 1) * P, n * K_INCH:(n + 1) * K_INCH],
            )
    for s in range(SHALF // K_INCH):
        for e in range(KC):
            nc.sync.dma_start(
                y_t[e][:, s * K_INCH:(s + 1) * K_INCH],
                yT[e * P:(e + 1) * P, s * K_INCH:(s + 1) * K_INCH],
            )

    # Stage 1 chunk emitter: Q2T[e, t] = sum_c M[c,e] xT[c,t] + v[e]
    def stage1(n):
        if K_MMORD == "es":
            for e in range(KC):
                ps = psum.tile(
                    [P, FD], mybir.dt.float32, tag=f"ps{e}", name="ps", bufs=2
                )
                for c in range(KC):
                    nc.tensor.matmul(
                        ps[:],
                        m_t[c][:, e * P:(e + 1) * P],
                        x_t[c][:, n * FD:(n + 1) * FD],
                        start=(c == 0),
                        stop=(c == KC - 1),
                    )
                if K_EVICT == "act" or (K_EVICT == "split" and e % 2 == 0):
                    nc.scalar.activation(
                        q2_t[e][:, n * FD:(n + 1) * FD], ps[:], ident,
                        bias=v_t[:, e:e + 1],
                    )
                else:
                    nc.vector.tensor_scalar_add(
                        q2_t[e][:, n * FD:(n + 1) * FD], ps[:], v_t[:, e:e + 1]
                    )
            return
        for e in range(KC):
            ps = psum.tile([P, FD], mybir.dt.float32, tag="ps", name="ps", bufs=7)
            for c in range(KC):
                nc.tensor.matmul(
                    ps[:],
                    m_t[c][:, e * P:(e + 1) * P],
                    x_t[c][:, n * FD:(n + 1) * FD],
                    start=(c == 0),
                    stop=(c == KC - 1),
                )
            # eviction rounds to f32r for the stage-2 matmul; alternate engines
            if K_EVICT == "act" or (K_EVICT == "split" and e % 2 == 0):
                nc.scalar.activation(
                    q2_t[e][:, n * FD:(n + 1) * FD], ps[:], ident, bias=v_t[:, e:e + 1]
                )
            else:
                nc.vector.tensor_scalar_add(
                    q2_t[e][:, n * FD:(n + 1) * FD], ps[:], v_t[:, e:e + 1]
                )

    out_eng = nc.scalar if K_OUT_RING == "act" else nc.sync

    # Stage 2: out[t, s] = sum_e Q2T[e,t] yT[e,s] + w[t].
    # Two PE orders:
    #  ei: per (tt, s) accumulate over e (both matmul operands advance each
    #      MM), processed in s-blocks of K_SBLK so the first pass needs only
    #      part of y.
    #  es: per tt, e-outer / s-inner across 4 parallel PSUM banks — the
    #      stationary operand q2[e][:,tt] is reused by 4 consecutive MMs,
    #      which measures ~25% faster per MM on HW.
    def stage2_tile(sb, tt):
        ot = ostage.tile([P, K_SBLK * FD], mybir.dt.float16, tag="ot", name="ot")
        for j in range(K_SBLK):
            s = sb * K_SBLK + j
            ps = psum.tile([P, FD], mybir.dt.float32, tag="ps", name="ps", bufs=7)
            for e in range(KC):
                nc.tensor.matmul(
                    ps[:],
                    q2_t[e][:, tt * P:(tt + 1) * P],
                    y_t[e][:, s * FD:(s + 1) * FD],
                    start=(e == 0),
                    stop=(e == KC - 1),
                )
            if K_EVICT == "act" or (K_EVICT == "split" and (tt + s) % 2 == 0):
                nc.scalar.activation(
                    ot[:, j * FD:(j + 1) * FD], ps[:], ident, bias=w_t[:, tt:tt + 1]
                )
            else:
                nc.vector.tensor_scalar_add(
                    ot[:, j * FD:(j + 1) * FD], ps[:], w_t[:, tt:tt + 1]
                )
        last = sb == NS_CHUNKS // K_SBLK - 1 and tt == NT_TILES - 1
        if not K_NO_OUT or last:
            out_eng.dma_start(
                out[tt * P:(tt + 1) * P, sb * K_SBLK * FD:(sb + 1) * K_SBLK * FD],
                ot[:],
            )

    def stage2_tile_es(tt):
        ot = ostage.tile([P, SHALF], mybir.dt.float16, tag="ot", name="ot")
        pss = [
            psum.tile([P, FD], mybir.dt.float32, tag=f"ps{s}", name=f"ps{s}", bufs=2)
            for s in range(NS_CHUNKS)
        ]
        for e in range(KC):
            for s in range(NS_CHUNKS):
                nc.tensor.matmul(
                    pss[s][:],
                    q2_t[e][:, tt * P:(tt + 1) * P],
                    y_t[e][:, s * FD:(s + 1) * FD],
                    start=(e == 0),
                    stop=(e == KC - 1),
                )
        for s in range(NS_CHUNKS):
            if K_EVICT == "act" or (K_EVICT == "split" and (tt + s) % 2 == 0):
                nc.scalar.activation(
                    ot[:, s * FD:(s + 1) * FD], pss[s][:], ident,
                    bias=w_t[:, tt:tt + 1],
                )
            else:
                nc.vector.tensor_scalar_add(
                    ot[:, s * FD:(s + 1) * FD], pss[s][:], w_t[:, tt:tt + 1]
                )
        if not K_NO_OUT or tt == NT_TILES - 1:
            out_eng.dma_start(out[tt * P:(tt + 1) * P, :], ot[:])

    # PE program order: all of stage 1 (its span covers the y DMA window),
    # then stage 2.
    for n in range(NT_CHUNKS):
        stage1(n)
    if K_MMORD == "es":
        for tt in range(NT_TILES):
            stage2_tile_es(tt)
    else:
        for sb in range(NS_CHUNKS // K_SBLK):
            for tt in range(NT_TILES):
                stage2_tile(sb, tt)


def _build(reps: int = 1, loop_reps: int = 1):
    """Build + compile the per-core Bass program. reps>1 statically unrolls
    the whole body; loop_reps>1 wraps it in a runtime For_i loop (both are
    used only for timing measurements)."""
    key = (reps, loop_reps)
    if key in _NC_CACHE:
        return _NC_CACHE[key]
    nc = bacc.Bacc(trn_type="TRN2", target_bir_lowering=False, debug=False)
    f32 = mybir.dt.float32
    f16 = mybir.dt.float16
    xT = nc.dram_tensor("xT", [D, T], f16, kind="ExternalInput").ap()
    yT = nc.dram_tensor("yT", [D, SHALF], f16, kind="ExternalInput").ap()
    m = nc.dram_tensor("m", [D, D], f16, kind="ExternalInput").ap()
    v = nc.dram_tensor("v", [P, KC], f32, kind="ExternalInput").ap()
    w = nc.dram_tensor("w", [P, NT_TILES], f32, kind="ExternalInput").ap()
    out = nc.dram_tensor("out", [T, SHALF], f16, kind="ExternalOutput").ap()
    with tile.TileContext(nc) as tc:
        with ExitStack() as ctx:
            tiles = _alloc(ctx, tc)
            if loop_reps > 1:
                hint = (
                    mybir.EngineType.PE,
                    mybir.EngineType.Activation,
                    mybir.EngineType.DVE,
                    mybir.EngineType.SP,
                )
                with tc.For_i(0, loop_reps, 1, hint_engines=hint):
                    for _ in range(reps):
                        _emit_body(tiles, tc, xT, yT, m, v, w, out)
            else:
                for _ in range(reps):
                    _emit_body(tiles, tc, xT, yT, m, v, w, out)
    nc.compile()
    _NC_CACHE[key] = nc
    return nc


def _host_prep(query, keys, q_w, q_b, k_w, k_b):
    """Fold weights/biases on host (float64), build per-core input maps."""
    q_w64 = np.asarray(q_w, np.float64)
    k_w64 = np.asarray(k_w, np.float64)
    q_b64 = np.asarray(q_b, np.float64)
    k_b64 = np.asarray(k_b, np.float64)

    m_in = np.ascontiguousarray(((q_w64.T @ k_w64) / SCALE).astype(np.float32))
    v64 = (k_w64.T @ q_b64) / SCALE  # [D]
    v_in = np.ascontiguousarray(v64.astype(np.float32).reshape(KC, P).T)
    g = q_w64.T @ k_b64  # [D]
    cc = float(q_b64 @ k_b64)
    # w[b, t] = (query[b] @ g + bq.bk) / 64
    w_all = ((np.asarray(query, np.float64) @ g + cc) / SCALE).astype(np.float32)

    m16 = np.ascontiguousarray(m_in.astype(np.float16))
    in_maps = []
    for i in range(N_CORES):
        b, h = divmod(i, N_CORES // B)
        in_maps.append(
            {
                "xT": np.ascontiguousarray(query[b].T.astype(np.float16)),
                "yT": np.ascontiguousarray(
                    keys[b, h * SHALF:(h + 1) * SHALF, :].T.astype(np.float16)
                ),
                "m": m16,
                "v": v_in,
                "w": np.ascontiguousarray(w_all[b].reshape(NT_TILES, P).T),
            }
        )
    return in_maps


def _gather(results, mask):
    out = np.empty((B, T, S), np.float32)
    for i in range(N_CORES):
        b, h = divmod(i, N_CORES // B)
        out[b, :, h * SHALF:(h + 1) * SHALF] = results[i]["out"].astype(np.float32)
    if mask is not None and mask.any():
        out = np.where(mask[:, None, :], np.float32(-np.inf), out)
    return out


def kernel(query, keys, key_padding_mask, q_w, q_b, k_w, k_b):
    query = np.asarray(query, np.float32)
    keys = np.asarray(keys, np.float32)
    mask = np.asarray(key_padding_mask, bool)
    assert query.shape == (B, T, D) and keys.shape == (B, S, D)

    in_maps = _host_prep(query, keys, q_w, q_b, k_w, k_b)
    nc = _build(reps=1)
    res = run_bass_kernel_spmd(nc, in_maps, core_ids=list(range(N_CORES)))
    return _gather(res.results, mask)



# revision 9
# speedup vs baseline: 1.1941x; 1.0646x over previous
"""Trainium2 Bass kernel for nn_PointerAttention (head-mean pointer logits).

Reference computation (B=4, T=2048, S=4096, D=512, H=8, HD=64):
    q = query @ q_w.T + q_b
    k = keys  @ k_w.T + k_b
    logits[b,t,s] = sum_d q[b,t,d] * k[b,s,d] / (H * sqrt(HD))   # = /64
    logits = where(mask[b,s], -inf, logits)

Algebraic refactor (all folding done on host in float64):
    Q = X Wq^T + 1 bq^T ;  K = Y Wk^T + 1 bk^T
    Q K^T = X (Wq^T Wk) Y^T + 1 (Y Wk^T bq)^T + (X Wq^T bk + bq.bk) 1^T
    Let  M = Wq^T Wk / 64          [D, D]
         v = Wk^T bq / 64          [D]     (per-partition bias of stage 1)
         w = (X (Wq^T bk) + bq.bk)/64  [T] per batch (per-partition bias, stage 2)
    Then out = (X M + 1 v^T) Y^T + w 1^T
       stage 1 (device): Q2T[e,t] = sum_c M[c,e] xT[c,t] + v[e]
       stage 2 (device): out[t,s] = sum_e Q2T[e,t] yT[e,s] + w[t]
    where xT = query[b].T and yT = keys[b].T are RAW inputs — only one
    projection-sized matmul remains and the K-side projection disappears.

Sharding: 8 cores = 4 batches x 2 T-halves (NOT S-halves): each core
computes out[b, thalf, :] = [1024, 4096]. T-sharding halves per-core
stage-1 PE work vs S-sharding (S-halved cores would each redo the full
X M projection); the cost is that both cores of a batch load the full
yT (4 MiB fp16) — cheap, since the kernel is PE-bound after fp16.

The whole device pipeline runs in float16 (inputs, Q2 intermediate, and
output; PSUM accumulation stays f32): fp16 matmul is 1 cycle/row on the
TRN2 PE (same as f32r) and halves DMA bytes. fp16's 10-bit mantissa
keeps rel-to-scale error ~5e-4 (measured vs f64), far inside the 2e-2
gate. The host upcasts the fp16 output to f32 after gather.

Stage-2 PE order: per (s-block, t-tile), e-outer / s-inner across 4
parallel PSUM banks — the stationary operand q2[e][:,tt] is reused by 4
consecutive MMs (measures ~25% faster per MM on HW than flipping the
loops). s-blocks are outermost so the first stage-2 tile only needs the
first half of y on SBUF.
"""

import os
from contextlib import ExitStack

import numpy as np

import concourse.bass as bass  # noqa: F401  (bass types used via tile/bacc)
import concourse.tile as tile
from concourse import bacc, mybir
from concourse.bass_utils import run_bass_kernel_spmd

# Problem dims (hardcoded; harness contract)
B, T, S, D = 4, 2048, 4096, 512
SCALE = 64.0  # N_HEADS * sqrt(HEAD_DIM) = 8 * 8
N_CORES = 8
TC = T // 2  # t rows per core (1024)
P = 128  # SBUF partitions
FD = 512  # matmul moving free dim == one fp32 PSUM bank
KC = D // P  # contraction chunks (4)
NT_TILES = TC // P  # output row tiles per core (8)
NT_CHUNKS = TC // FD  # stage-1 moving chunks (2)
NS_CHUNKS = S // FD  # output col chunks per core (8)

_NC_CACHE: dict = {}

# experiment toggles (timing A/B only; defaults are the shipped config)
K_WARMUP = os.environ.get("K_WARMUP", "0") == "1"
K_OUT_RING = os.environ.get("K_OUT_RING", "act")
K_NO_OUT = os.environ.get("K_NO_OUT", "0") == "1"   # timing ablation only
K_EVICT = os.environ.get("K_EVICT", "split")        # split | act | dve
K_SBLK = int(os.environ.get("K_SBLK", "4"))  # s-chunks per stage-2 block
K_INCH = int(os.environ.get("K_INCH", "512"))  # input DMA chunk columns
NS_BLOCKS = NS_CHUNKS // K_SBLK


def _alloc(ctx: ExitStack, tc):
    f32 = mybir.dt.float32
    f16 = mybir.dt.float16
    persist = ctx.enter_context(tc.tile_pool(name="persist", bufs=1))
    psum = ctx.enter_context(tc.tile_pool(name="psum", bufs=8, space="PSUM"))
    ostage = ctx.enter_context(tc.tile_pool(name="ostage", bufs=6))
    tiles = {
        "psum": psum,
        "ostage": ostage,
        "m": [persist.tile([P, D], f16, tag=f"m{c}", name=f"m{c}") for c in range(KC)],
        "x": [persist.tile([P, TC], f16, tag=f"x{c}", name=f"x{c}") for c in range(KC)],
        "y": [
            persist.tile([P, S], f16, tag=f"y{e}", name=f"y{e}")
            for e in range(KC)
        ],
        "q2": [
            persist.tile([P, TC], f16, tag=f"q2{e}", name=f"q2{e}") for e in range(KC)
        ],
        "v": persist.tile([P, KC], f32, tag="v", name="vt"),
        "w": persist.tile([P, NT_TILES], f32, tag="w", name="wt"),
        "warm": persist.tile([P, 256], f16, tag="warm", name="warm"),
        "warm_f32": persist.tile([P, 256], f32, tag="warm_f32", name="warm_f32"),
    }
    return tiles


def _emit_body(tiles, tc, xT, yT, m, v, w, out):
    nc = tc.nc
    ident = mybir.ActivationFunctionType.Identity
    psum, ostage = tiles["psum"], tiles["ostage"]
    m_t, x_t, y_t, q2_t = tiles["m"], tiles["x"], tiles["y"], tiles["q2"]
    v_t, w_t = tiles["v"], tiles["w"]

    nc.sync.dma_start(v_t[:], v[:])
    nc.sync.dma_start(w_t[:], w[:])

    # PE warmup: ~16 junk matmuls during the initial DMA wait so the HAM
    # clock-gate reaches 8/8 before the first real matmul.
    if K_WARMUP:
        warm = tiles["warm"]
        warm_f32 = tiles["warm_f32"]
        wps = tiles["psum"].tile(
            [P, 256], mybir.dt.float32, tag="wps", name="wps", bufs=1
        )
        nc.vector.memset(warm_f32[:], 0.0)
        nc.vector.tensor_copy(warm[:], warm_f32[:])
        for i in range(16):
            nc.tensor.matmul(
                wps[:], warm[:, 0:P], warm[:], start=(i == 0), stop=(i == 15)
            )

    # Input loads, in consumption order: M (stage-1 needs all of it first),
    # x in stage-1 n-order, then y in stage-2 block order (all e-tiles of
    # s-block 0 before s-block 1). Inputs ride the SP HWDGE ring; outputs
    # ride the ACT ring (separate FIFO, no head-of-line blocking).
    for c in range(KC):
        nc.sync.dma_start(m_t[c][:], m[c * P:(c + 1) * P, :])
    for n in range(TC // K_INCH):
        for c in range(KC):
            nc.sync.dma_start(
                x_t[c][:, n * K_INCH:(n + 1) * K_INCH],
                xT[c * P:(c + 1) * P, n * K_INCH:(n + 1) * K_INCH],
            )
    for s in range(S // K_INCH):
        for e in range(KC):
            nc.sync.dma_start(
                y_t[e][:, s * K_INCH:(s + 1) * K_INCH],
                yT[e * P:(e + 1) * P, s * K_INCH:(s + 1) * K_INCH],
            )

    # Stage 1: Q2T[e, t] = sum_c M[c,e] xT[c,t] + v[e], per t-chunk n.
    def stage1(n):
        for e in range(KC):
            ps = psum.tile(
                [P, FD], mybir.dt.float32, tag=f"ps{e}", name="ps", bufs=2
            )
            for c in range(KC):
                nc.tensor.matmul(
                    ps[:],
                    m_t[c][:, e * P:(e + 1) * P],
                    x_t[c][:, n * FD:(n + 1) * FD],
                    start=(c == 0),
                    stop=(c == KC - 1),
                )
            # eviction rounds to fp16 for the stage-2 matmul; alternate engines
            if K_EVICT == "act" or (K_EVICT == "split" and e % 2 == 0):
                nc.scalar.activation(
                    q2_t[e][:, n * FD:(n + 1) * FD], ps[:], ident,
                    bias=v_t[:, e:e + 1],
                )
            else:
                nc.vector.tensor_scalar_add(
                    q2_t[e][:, n * FD:(n + 1) * FD], ps[:], v_t[:, e:e + 1]
                )

    out_eng = nc.scalar if K_OUT_RING == "act" else nc.sync

    # Stage 2: out[t, s] = sum_e Q2T[e,t] yT[e,s] + w[t], one (sb, tt) pass
    # covers s-chunks [sb*K_SBLK, (sb+1)*K_SBLK) across K_SBLK PSUM banks.
    def stage2_tile(sb, tt):
        ot = ostage.tile([P, K_SBLK * FD], mybir.dt.float16, tag="ot", name="ot")
        pss = [
            psum.tile([P, FD], mybir.dt.float32, tag=f"ps{j}", name=f"ps{j}", bufs=2)
            for j in range(K_SBLK)
        ]
        for e in range(KC):
            for j in range(K_SBLK):
                s = sb * K_SBLK + j
                nc.tensor.matmul(
                    pss[j][:],
                    q2_t[e][:, tt * P:(tt + 1) * P],
                    y_t[e][:, s * FD:(s + 1) * FD],
                    start=(e == 0),
                    stop=(e == KC - 1),
                )
        for j in range(K_SBLK):
            if K_EVICT == "act" or (K_EVICT == "split" and (tt + j) % 2 == 0):
                nc.scalar.activation(
                    ot[:, j * FD:(j + 1) * FD], pss[j][:], ident,
                    bias=w_t[:, tt:tt + 1],
                )
            else:
                nc.vector.tensor_scalar_add(
                    ot[:, j * FD:(j + 1) * FD], pss[j][:], w_t[:, tt:tt + 1]
                )
        last = sb == NS_BLOCKS - 1 and tt == NT_TILES - 1
        if not K_NO_OUT or last:
            out_eng.dma_start(
                out[tt * P:(tt + 1) * P, sb * K_SBLK * FD:(sb + 1) * K_SBLK * FD],
                ot[:],
            )

    # PE program order: stage 1 (covers the y s-block-0 DMA window), then
    # stage 2 s-block by s-block.
    for n in range(NT_CHUNKS):
        stage1(n)
    for sb in range(NS_BLOCKS):
        for tt in range(NT_TILES):
            stage2_tile(sb, tt)


def _build(reps: int = 1, loop_reps: int = 1):
    """Build + compile the per-core Bass program. reps>1 statically unrolls
    the whole body; loop_reps>1 wraps it in a runtime For_i loop (both are
    used only for timing measurements)."""
    key = (reps, loop_reps)
    if key in _NC_CACHE:
        return _NC_CACHE[key]
    nc = bacc.Bacc(trn_type="TRN2", target_bir_lowering=False, debug=False)
    f32 = mybir.dt.float32
    f16 = mybir.dt.float16
    xT = nc.dram_tensor("xT", [D, TC], f16, kind="ExternalInput").ap()
    yT = nc.dram_tensor("yT", [D, S], f16, kind="ExternalInput").ap()
    m = nc.dram_tensor("m", [D, D], f16, kind="ExternalInput").ap()
    v = nc.dram_tensor("v", [P, KC], f32, kind="ExternalInput").ap()
    w = nc.dram_tensor("w", [P, NT_TILES], f32, kind="ExternalInput").ap()
    out = nc.dram_tensor("out", [TC, S], f16, kind="ExternalOutput").ap()
    with tile.TileContext(nc) as tc:
        with ExitStack() as ctx:
            tiles = _alloc(ctx, tc)
            if loop_reps > 1:
                hint = (
                    mybir.EngineType.PE,
                    mybir.EngineType.Activation,
                    mybir.EngineType.DVE,
                    mybir.EngineType.SP,
                )
                with tc.For_i(0, loop_reps, 1, hint_engines=hint):
                    for _ in range(reps):
                        _emit_body(tiles, tc, xT, yT, m, v, w, out)
            else:
                for _ in range(reps):
                    _emit_body(tiles, tc, xT, yT, m, v, w, out)
    nc.compile()
    _NC_CACHE[key] = nc
    return nc


def _host_prep(query, keys, q_w, q_b, k_w, k_b):
    """Fold weights/biases on host (float64), build per-core input maps."""
    q_w64 = np.asarray(q_w, np.float64)
    k_w64 = np.asarray(k_w, np.float64)
    q_b64 = np.asarray(q_b, np.float64)
    k_b64 = np.asarray(k_b, np.float64)

    m_in = np.ascontiguousarray(((q_w64.T @ k_w64) / SCALE).astype(np.float16))
    v64 = (k_w64.T @ q_b64) / SCALE  # [D]
    v_in = np.ascontiguousarray(v64.astype(np.float32).reshape(KC, P).T)
    g = q_w64.T @ k_b64  # [D]
    cc = float(q_b64 @ k_b64)
    # w[b, t] = (query[b] @ g + bq.bk) / 64
    w_all = ((np.asarray(query, np.float64) @ g + cc) / SCALE).astype(np.float32)

    yT16 = [np.ascontiguousarray(keys[b].T.astype(np.float16)) for b in range(B)]
    in_maps = []
    for i in range(N_CORES):
        b, th = divmod(i, N_CORES // B)
        tsl = slice(th * TC, (th + 1) * TC)
        in_maps.append(
            {
                "xT": np.ascontiguousarray(query[b, tsl].T.astype(np.float16)),
                "yT": yT16[b],
                "m": m_in,
                "v": v_in,
                "w": np.ascontiguousarray(
                    w_all[b, tsl].reshape(NT_TILES, P).T
                ),
            }
        )
    return in_maps


def _gather(results, mask):
    out = np.empty((B, T, S), np.float32)
    for i in range(N_CORES):
        b, th = divmod(i, N_CORES // B)
        out[b, th * TC:(th + 1) * TC, :] = results[i]["out"].astype(np.float32)
    if mask is not None and mask.any():
        out = np.where(mask[:, None, :], np.float32(-np.inf), out)
    return out


def kernel(query, keys, key_padding_mask, q_w, q_b, k_w, k_b):
    query = np.asarray(query, np.float32)
    keys = np.asarray(keys, np.float32)
    mask = np.asarray(key_padding_mask, bool)
    assert query.shape == (B, T, D) and keys.shape == (B, S, D)

    in_maps = _host_prep(query, keys, q_w, q_b, k_w, k_b)
    nc = _build(reps=1)
    res = run_bass_kernel_spmd(nc, in_maps, core_ids=list(range(N_CORES)))
    return _gather(res.results, mask)
